# revision 3
# baseline (speedup 1.0000x reference)
"""Multi-head attention (B=2, S=2048, D=1024, H=16) on 8 NeuronCores.

Sharding: core c -> batch b = c//4, head group g = c%4 (4 heads each).
Each core computes q/k/v projections for its head group, full softmax
attention for its 4 heads, and a partial output projection
out_c = attn_out_c @ Wo[rows_c].  The host sums the 4 partials per batch
and adds bo.

v2 pipeline (single fused emission, ScalarE-exp is the bottleneck engine):
  - phase 1 (QKV, KC=8 chunks of 128): per 512-wide s-chunk, qT/kT computed
    in transposed layout (head pair on partitions), v in natural layout with
    a ones column (v_aug) so the attention matmul accumulates the softmax
    denominator as row 64.  Biases added on DVE during PSUM evacuation.
  - attention groups (hp, sqc): scoresT = k @ qT per 128-k-block (two heads
    row-tiled concurrently), exp on ScalarE out of PSUM, outT accumulation
    in PSUM over the 16 k-blocks.  Group (hp0, sqc0) is interleaved into
    phase 1 so the ScalarE starts early; the remaining 7 groups stream
    back-to-back, keeping ScalarE ~saturated.
  - group epilogue (off critical path, pipelined 1-2 groups deep):
    un-normalized rows copied to a staging tile (frees the PSUM
    accumulators immediately), denominator reciprocals via
    reciprocal_approx_fast, broadcast across partitions via a DRAM bounce,
    normalization multiply on the staging tile, then SBUF->SBUF DMAs
    pair-pack the two heads onto 128 partitions of outTs.
  - projection: K=128 matmuls (head-pair packed), interleaved as filler
    work into the next sqc's attention stream so ScalarE never starves.
"""

import numpy as np

S = 2048
D = 1024
H = 16
DEPTH = 64
NCORES = 8
GH = 4              # heads per core
GD = GH * DEPTH     # 256 output dims per core
KC = 8              # contraction chunks of 128 (1024 = D)

_state = {}


def _build():
    import concourse.mybir as mybir
    import concourse.tile as tile
    from concourse import bacc
    from concourse.bass import ts

    fp32 = mybir.dt.float32
    # All matmul operands live as float32r (same 4-byte layout, np.float32 on
    # the host): the PE streams fp32r at 1 col/cycle vs fp32's 4, at ~tf32
    # operand precision.  PSUM accumulation stays fp32.
    fp32r = mybir.dt.float32r
    Exp = mybir.ActivationFunctionType.Exp
    Add = mybir.AluOpType.add
    Mult = mybir.AluOpType.mult

    nc = bacc.Bacc("TRN2", target_bir_lowering=False, debug=False)
    xT = nc.dram_tensor("xT", [D, S], fp32r, kind="ExternalInput")
    wq = nc.dram_tensor("wq", [D, GD], fp32r, kind="ExternalInput")
    wk = nc.dram_tensor("wk", [D, GD], fp32r, kind="ExternalInput")
    wv = nc.dram_tensor("wv", [D, GD], fp32r, kind="ExternalInput")
    bq = nc.dram_tensor("bq", [GD], fp32, kind="ExternalInput")
    bk = nc.dram_tensor("bk", [GD], fp32, kind="ExternalInput")
    bv = nc.dram_tensor("bv", [GD], fp32, kind="ExternalInput")
    wo = nc.dram_tensor("wo", [GD, D], fp32r, kind="ExternalInput")
    out = nc.dram_tensor("out", [S, D], fp32, kind="ExternalOutput")
    # per-group softmax denominator reciprocals bounce through DRAM to get
    # partition-broadcast on the way back in
    rec_dram = nc.dram_tensor("denom_rec", [8 * 1024], fp32)

    with tile.TileContext(nc) as tc:
        with (
            tc.tile_pool(name="singles", bufs=1) as singles,
            tc.tile_pool(name="xpool", bufs=2) as xpool,
            tc.tile_pool(name="expp", bufs=3) as expp,
            tc.tile_pool(name="stp", bufs=4) as stp,
            tc.tile_pool(name="rqp", bufs=2) as rqp,
            tc.tile_pool(name="rbp", bufs=2) as rbp,
            tc.tile_pool(name="otp", bufs=2) as otp,
            tc.tile_pool(name="paux", bufs=2, space="PSUM") as paux,
            tc.tile_pool(name="pss", bufs=2, space="PSUM") as pss,
            tc.tile_pool(name="poab", bufs=2, space="PSUM") as poab,
        ):
            qT = singles.tile([128, 2, S], fp32r)       # [dout%128, pair, sq]
            kT = singles.tile([128, 2, S], fp32r)
            v_sb = singles.tile([128, 16, GH, DEPTH + 1], fp32r)  # v_aug
            outTs = singles.tile([128, 2, S], fp32r)    # pair-packed attn out
            wq_sb = singles.tile([128, KC, GD], fp32r)
            wk_sb = singles.tile([128, KC, GD], fp32r)
            wv_sb = singles.tile([128, KC, GD], fp32r)
            wo_sb = singles.tile([128, 2, D], fp32r)    # pair-packed Wo rows
            bq_sb = singles.tile([128, 2], fp32)        # [dout%128, pair]
            bk_sb = singles.tile([128, 2], fp32)
            bv_sb = singles.tile([128, GH, DEPTH], fp32)  # partition-bcast
            scr = singles.tile([1, 16], fp32)

            nc.vector.memset(v_sb[:, :, :, DEPTH : DEPTH + 1].bitcast(fp32), 1.0)
            # prime the ScalarE exp table load during the DMA prologue
            nc.vector.memset(scr[:], 0.0)
            nc.scalar.activation(scr[0:1, 8:16], scr[0:1, 0:8], Exp)

            # ---------------- input DMAs (priority ~ emission order) -------
            nc.sync.dma_start(
                wq_sb[:], wq[:].rearrange("(c p) d -> p c d", p=128)
            )
            nc.sync.dma_start(
                wk_sb[:], wk[:].rearrange("(c p) d -> p c d", p=128)
            )
            xT_view = xT[:].rearrange("(c p) s -> p c s", p=128)
            nc.sync.dma_start(
                wv_sb[:], wv[:].rearrange("(c p) d -> p c d", p=128)
            )
            nc.sync.dma_start(bq_sb[:], bq[:].rearrange("(hp p) -> p hp", p=128))
            nc.sync.dma_start(bk_sb[:], bk[:].rearrange("(hp p) -> p hp", p=128))
            nc.sync.dma_start(
                bv_sb[:],
                bv[:].rearrange("(p h d) -> p h d", p=1, h=GH).to_broadcast(
                    [128, GH, DEPTH]
                ),
            )
            nc.sync.dma_start(
                wo_sb[:], wo[:].rearrange("(hp p) n -> p hp n", p=128)
            )

            # ---------------- emission helpers ----------------------------
            def emit_sc(sc):
                """Phase-1 chunk: q/k/v projections for s-columns [512sc,512sc+512)."""
                xc = xpool.tile([128, KC, 512], fp32r, tag="xc")
                nc.sync.dma_start(xc[:], xT_view[:, :, ts(sc, 512)])
                for hp in range(2):
                    for w_sb, dstT, b_sb in (
                        (wq_sb, qT, bq_sb),
                        (wk_sb, kT, bk_sb),
                    ):
                        ps = paux.tile([128, 512], fp32, tag="aux")
                        for kc in range(KC):
                            nc.tensor.matmul(
                                ps[:],
                                w_sb[:, kc, ts(hp, 128)],
                                xc[:, kc, :],
                                start=(kc == 0),
                                stop=(kc == KC - 1),
                            )
                        nc.vector.tensor_tensor(
                            dstT[:, hp, ts(sc, 512)],
                            ps[:],
                            b_sb[:, hp, None].to_broadcast([128, 512]),
                            Add,
                        )
                for mm in range(4):
                    ps = paux.tile([128, GD], fp32, tag="aux")
                    for kc in range(KC):
                        nc.tensor.matmul(
                            ps[:],
                            xc[:, kc, ts(mm, 128)],
                            wv_sb[:, kc, :],
                            start=(kc == 0),
                            stop=(kc == KC - 1),
                        )
                    nc.vector.tensor_tensor(
                        v_sb[:, sc * 4 + mm, :, 0:DEPTH],
                        ps[:].rearrange("p (h d) -> p h d", h=GH),
                        bv_sb[:],
                        Add,
                    )

            def emit_kb(hp, sqc, kb, oab):
                """One attention k-block: scoresT pair -> exp -> outT accum."""
                sps = pss.tile([128, 2, 512], fp32, tag="s")
                for a in range(2):
                    nc.tensor.matmul(
                        sps[:, a, :],
                        kT[a * 64 : (a + 1) * 64, hp, ts(kb, 128)],
                        qT[a * 64 : (a + 1) * 64, hp, ts(sqc, 512)],
                        start=True,
                        stop=True,
                    )
                ex = expp.tile([128, 2, 512], fp32r, tag="e")
                nc.scalar.activation(ex[:], sps[:], Exp, scale=0.125)
                for a in range(2):
                    nc.tensor.matmul(
                        oab[a][:],
                        v_sb[:, kb, 2 * hp + a, :],
                        ex[:, a, :],
                        start=(kb == 0),
                        stop=(kb == 15),
                    )

            def endgroup(hp, sqc, oab, g):
                """Evacuate + normalize one finished group (off critical path)."""
                stage = stp.tile([64, 2, 512], fp32r, tag="st")
                recq = rqp.tile([65, 1024], fp32, tag="rq")
                for a in range(2):
                    nc.vector.tensor_copy(stage[:, a, :], oab[a][0:64, :])
                    # raw denominator row -> SBUF (DVE can't shift partitions
                    # and DMA can't read PSUM, so it stays on partition 64)
                    nc.vector.tensor_copy(
                        recq[64:65, ts(a, 512)], oab[a][64:65, :]
                    )
                nc.sync.dma_start(
                    rec_dram[g * 1024 : (g + 1) * 1024].rearrange(
                        "(p x) -> p x", p=1
                    ),
                    recq[64:65, :],
                )
                rbt = rbp.tile([64, 2, 1024], fp32, tag="rb")
                nc.sync.dma_start(
                    rbt[:, 0, :],
                    rec_dram[g * 1024 : (g + 1) * 1024]
                    .rearrange("(p x) -> p x", p=1)
                    .to_broadcast([64, 1024]),
                )
                nc.vector.reciprocal_approx_fast(rbt[:, 1, :], rbt[:, 0, :])
                for a in range(2):
                    nc.vector.tensor_tensor(
                        stage[:, a, :],
                        stage[:, a, :],
                        rbt[:, 1, ts(a, 512)].bitcast(fp32r),
                        Mult,
                    )
                    nc.sync.dma_start(
                        outTs[a * 64 : (a + 1) * 64, hp, ts(sqc, 512)],
                        stage[:, a, :],
                    )

            def proj_pieces(sqc):
                """Output projection for one sqc as a list of emission thunks."""
                pieces = []
                for mm in range(4):
                    m = sqc * 4 + mm
                    for nn in range(2):
                        def piece(m=m, nn=nn):
                            ps = paux.tile([128, 512], fp32, tag="aux")
                            for hp in range(2):
                                nc.tensor.matmul(
                                    ps[:],
                                    outTs[:, hp, ts(m, 128)],
                                    wo_sb[:, hp, ts(nn, 512)],
                                    start=(hp == 0),
                                    stop=(hp == 1),
                                )
                            ot = otp.tile([128, 512], fp32, tag="ot")
                            nc.vector.tensor_copy(ot[:], ps[:])
                            nc.sync.dma_start(
                                out[m * 128 : (m + 1) * 128, ts(nn, 512)],
                                ot[:],
                            )
                        pieces.append(piece)
                return pieces

            def emit_group(hp, sqc, g, fillers=()):
                """16 k-blocks + epilogue; fillers run every other k-block."""
                oab = [
                    poab.tile([65, 512], fp32, tag="o", name=f"o{a}")
                    for a in range(2)
                ]
                fill = list(fillers)
                fi = 0
                for kb in range(16):
                    emit_kb(hp, sqc, kb, oab)
                    if kb % 2 == 1 and fi < len(fill):
                        fill[fi]()
                        fi += 1
                while fi < len(fill):
                    fill[fi]()
                    fi += 1
                endgroup(hp, sqc, oab, g)

            # ---------------- main emission --------------------------------
            # group (hp0, sqc0) is interleaved into phase 1: its k-block j
            # only needs kT/v for s-chunk j//4, which emit_sc(j//4) provides.
            oab00 = [
                poab.tile([65, 512], fp32, tag="o", name=f"o{a}")
                for a in range(2)
            ]
            for sc in range(4):
                emit_sc(sc)
                for kb in range(4 * sc, 4 * sc + 4):
                    emit_kb(0, 0, kb, oab00)
            endgroup(0, 0, oab00, 0)

            emit_group(1, 0, 1)
            emit_group(0, 1, 2, proj_pieces(0))
            emit_group(1, 1, 3)
            emit_group(0, 2, 4, proj_pieces(1))
            emit_group(1, 2, 5)
            emit_group(0, 3, 6, proj_pieces(2))
            emit_group(1, 3, 7)
            for piece in proj_pieces(3):
                piece()

    nc.compile()
    return nc


def _get_nc():
    if "nc" not in _state:
        _state["nc"] = _build()
    return _state["nc"]


def _prep_core_inputs(inputs, Wq, bq, Wk, bk, Wv, bv, Wo, bo):
    """Build the 8 per-core input dicts (host-side shard + transpose)."""
    in_maps = []
    xTs = [np.ascontiguousarray(inputs[b].T, dtype=np.float32) for b in range(2)]
    for c in range(NCORES):
        b, g = divmod(c, 4)
        cols = slice(g * GD, (g + 1) * GD)
        m = {
            "xT": xTs[b],
            "wq": np.ascontiguousarray(Wq[:, cols], dtype=np.float32),
            "wk": np.ascontiguousarray(Wk[:, cols], dtype=np.float32),
            "wv": np.ascontiguousarray(Wv[:, cols], dtype=np.float32),
            "bq": np.ascontiguousarray(bq[cols], dtype=np.float32),
            "bk": np.ascontiguousarray(bk[cols], dtype=np.float32),
            "bv": np.ascontiguousarray(bv[cols], dtype=np.float32),
            "wo": np.ascontiguousarray(Wo[cols, :], dtype=np.float32),
        }
        in_maps.append(m)
    return in_maps


def run(inputs, Wq, bq, Wk, bk, Wv, bv, Wo, bo, trace=False):
    from concourse.bass_utils import run_bass_kernel_spmd

    nc = _get_nc()
    in_maps = _prep_core_inputs(inputs, Wq, bq, Wk, bk, Wv, bv, Wo, bo)
    res = run_bass_kernel_spmd(
        nc, in_maps, core_ids=list(range(NCORES)), trace=trace
    )
    out = np.zeros((2, S, D), np.float32)
    for c in range(NCORES):
        out[c // 4] += res.results[c]["out"]
    out += np.asarray(bo, np.float32)
    return out, res


def kernel(inputs, Wq, bq, Wk, bk, Wv, bv, Wo, bo):
    out, _ = run(
        np.asarray(inputs, np.float32),
        np.asarray(Wq, np.float32), np.asarray(bq, np.float32),
        np.asarray(Wk, np.float32), np.asarray(bk, np.float32),
        np.asarray(Wv, np.float32), np.asarray(bv, np.float32),
        np.asarray(Wo, np.float32), np.asarray(bo, np.float32),
    )
    return out


# revision 6
# speedup vs baseline: 1.0695x; 1.0695x over previous
"""Multi-head attention (B=2, S=2048, D=1024, H=16) on 8 NeuronCores.

Sharding: core c -> batch b = c//4, head group g = c%4 (4 heads each).
Each core computes q/k/v projections for its head group, full softmax
attention for its 4 heads, and a partial output projection
out_c = attn_out_c @ Wo[rows_c].  The host sums the 4 partials per batch
and adds bo.

v2 pipeline (single fused emission, ScalarE-exp is the bottleneck engine):
  - phase 1 (QKV, KC=8 chunks of 128): per 512-wide s-chunk, qT/kT computed
    in transposed layout (head pair on partitions), v in natural layout with
    a ones column (v_aug) so the attention matmul accumulates the softmax
    denominator as row 64.  Biases added on DVE during PSUM evacuation.
  - attention groups (hp, sqc): scoresT = k @ qT per 128-k-block (two heads
    row-tiled concurrently), exp on ScalarE out of PSUM, outT accumulation
    in PSUM over the 16 k-blocks.  Group (hp0, sqc0) is interleaved into
    phase 1 so the ScalarE starts early; the remaining 7 groups stream
    back-to-back, keeping ScalarE ~saturated.
  - group epilogue (off critical path, pipelined 1-2 groups deep):
    un-normalized rows copied to a staging tile (frees the PSUM
    accumulators immediately), denominator reciprocals via
    reciprocal_approx_fast, broadcast across partitions via a DRAM bounce,
    normalization multiply on the staging tile, then SBUF->SBUF DMAs
    pair-pack the two heads onto 128 partitions of outTs.
  - projection: K=128 matmuls (head-pair packed), interleaved as filler
    work into the next sqc's attention stream so ScalarE never starves.
"""

import numpy as np

S = 2048
D = 1024
H = 16
DEPTH = 64
NCORES = 8
GH = 4              # heads per core
GD = GH * DEPTH     # 256 output dims per core
KC = 8              # contraction chunks of 128 (1024 = D)

_state = {}


def _build():
    import concourse.mybir as mybir
    import concourse.tile as tile
    from concourse import bacc
    from concourse.bass import ts

    fp32 = mybir.dt.float32
    # All matmul operands live as float32r (same 4-byte layout, np.float32 on
    # the host): the PE streams fp32r at 1 col/cycle vs fp32's 4, at ~tf32
    # operand precision.  PSUM accumulation stays fp32.
    fp32r = mybir.dt.float32r
    Exp = mybir.ActivationFunctionType.Exp
    Add = mybir.AluOpType.add
    Mult = mybir.AluOpType.mult

    nc = bacc.Bacc("TRN2", target_bir_lowering=False, debug=False)
    xT = nc.dram_tensor("xT", [D, S], fp32r, kind="ExternalInput")
    wq = nc.dram_tensor("wq", [D, GD], fp32r, kind="ExternalInput")
    wk = nc.dram_tensor("wk", [D, GD], fp32r, kind="ExternalInput")
    wv = nc.dram_tensor("wv", [D, GD], fp32r, kind="ExternalInput")
    bq = nc.dram_tensor("bq", [GD], fp32, kind="ExternalInput")
    bk = nc.dram_tensor("bk", [GD], fp32, kind="ExternalInput")
    bv = nc.dram_tensor("bv", [GD], fp32, kind="ExternalInput")
    wo = nc.dram_tensor("wo", [GD, D], fp32r, kind="ExternalInput")
    out = nc.dram_tensor("out", [S, D], fp32, kind="ExternalOutput")

    with tile.TileContext(nc) as tc:
        with (
            tc.tile_pool(name="singles", bufs=1) as singles,
            tc.tile_pool(name="xpool", bufs=2) as xpool,
            tc.tile_pool(name="expp", bufs=3) as expp,
            tc.tile_pool(name="stp", bufs=4) as stp,
            tc.tile_pool(name="rqp", bufs=2) as rqp,
            tc.tile_pool(name="rbp", bufs=2) as rbp,
            tc.tile_pool(name="otp", bufs=4) as otp,
            tc.tile_pool(name="paux", bufs=2, space="PSUM") as paux,
            tc.tile_pool(name="pss", bufs=2, space="PSUM") as pss,
            tc.tile_pool(name="poab", bufs=2, space="PSUM") as poab,
        ):
            qT = singles.tile([128, 2, S], fp32r)       # [dout%128, pair, sq]
            kT = singles.tile([128, 2, S], fp32r)
            v_sb = singles.tile([128, 16, GH, DEPTH + 1], fp32r)  # v_aug (ones col 64)
            outTs = singles.tile([128, 2, S], fp32r)    # pair-packed attn out
            wq_sb = singles.tile([128, KC, GD], fp32r)
            wk_sb = singles.tile([128, KC, GD], fp32r)
            wv_sb = singles.tile([128, KC, GD], fp32r)
            wo_sb = singles.tile([128, 2, D], fp32r)    # pair-packed Wo rows
            bq_sb = singles.tile([128, 2], fp32)        # [dout%128, pair]
            bk_sb = singles.tile([128, 2], fp32)
            bv_sb = singles.tile([128, GH, DEPTH], fp32)  # partition-bcast
            scr = singles.tile([1, 16], fp32)

            nc.vector.memset(v_sb[:, :, :, DEPTH : DEPTH + 1].bitcast(fp32), 1.0)
            # prime the ScalarE exp table load during the DMA prologue
            nc.vector.memset(scr[:], 0.0)
            nc.scalar.activation(scr[0:1, 8:16], scr[0:1, 0:8], Exp)

            # ---------------- input DMAs (priority ~ emission order) -------
            # wq then x(sc0) first: the first q matmul group needs exactly
            # those two, so compute starts ~10us in while wk/wv stream behind.
            xT_view = xT[:].rearrange("(c p) s -> p c s", p=128)
            nc.sync.dma_start(
                wq_sb[:], wq[:].rearrange("(c p) d -> p c d", p=128)
            )
            xc0 = xpool.tile([128, KC, 512], fp32r, tag="xc")
            nc.sync.dma_start(xc0[:], xT_view[:, :, ts(0, 512)])
            nc.sync.dma_start(
                wk_sb[:], wk[:].rearrange("(c p) d -> p c d", p=128)
            )
            nc.sync.dma_start(
                wv_sb[:], wv[:].rearrange("(c p) d -> p c d", p=128)
            )
            nc.sync.dma_start(bq_sb[:], bq[:].rearrange("(hp p) -> p hp", p=128))
            nc.sync.dma_start(bk_sb[:], bk[:].rearrange("(hp p) -> p hp", p=128))
            nc.sync.dma_start(
                bv_sb[:],
                bv[:].rearrange("(p h d) -> p h d", p=1, h=GH).to_broadcast(
                    [128, GH, DEPTH]
                ),
            )

            # ---------------- emission helpers ----------------------------
            def emit_sc(sc, xc=None):
                """Phase-1 chunk: q/k/v projections for s-columns [512sc,512sc+512)."""
                if xc is None:
                    xc = xpool.tile([128, KC, 512], fp32r, tag="xc")
                    nc.sync.dma_start(xc[:], xT_view[:, :, ts(sc, 512)])
                for hp in range(2):
                    for w_sb, dstT, b_sb in (
                        (wq_sb, qT, bq_sb),
                        (wk_sb, kT, bk_sb),
                    ):
                        ps = paux.tile([128, 512], fp32, tag="aux")
                        for kc in range(KC):
                            nc.tensor.matmul(
                                ps[:],
                                w_sb[:, kc, ts(hp, 128)],
                                xc[:, kc, :],
                                start=(kc == 0),
                                stop=(kc == KC - 1),
                            )
                        nc.vector.tensor_tensor(
                            dstT[:, hp, ts(sc, 512)],
                            ps[:],
                            b_sb[:, hp, None].to_broadcast([128, 512]),
                            Add,
                        )
                for mm in range(4):
                    ps = paux.tile([128, GD], fp32, tag="aux")
                    for kc in range(KC):
                        nc.tensor.matmul(
                            ps[:],
                            xc[:, kc, ts(mm, 128)],
                            wv_sb[:, kc, :],
                            start=(kc == 0),
                            stop=(kc == KC - 1),
                        )
                    nc.vector.tensor_tensor(
                        v_sb[:, sc * 4 + mm, :, 0:DEPTH],
                        ps[:].rearrange("p (h d) -> p h d", h=GH),
                        bv_sb[:],
                        Add,
                    )

            def emit_kb(hp, sqc, kb, oab):
                """One attention k-block: scoresT pair -> exp -> outT accum."""
                sps = pss.tile([128, 2, 512], fp32, tag="s")
                for a in range(2):
                    nc.tensor.matmul(
                        sps[:, a, :],
                        kT[a * 64 : (a + 1) * 64, hp, ts(kb, 128)],
                        qT[a * 64 : (a + 1) * 64, hp, ts(sqc, 512)],
                        start=True,
                        stop=True,
                    )
                ex = expp.tile([128, 2, 512], fp32r, tag="e")
                nc.scalar.activation(ex[:], sps[:], Exp, scale=0.125)
                for a in range(2):
                    nc.tensor.matmul(
                        oab[a][:],
                        v_sb[:, kb, 2 * hp + a, :],
                        ex[:, a, :],
                        start=(kb == 0),
                        stop=(kb == 15),
                    )

            def endgroup(hp, sqc, oab, g):
                """Evacuate + normalize one finished group (off critical path).

                oab rows: 0..63 = attention output, 64 = softmax denominator
                (v_aug ones col).  Engine ops need 32-aligned base partitions
                and ucode ops need base 0, so: DVE-copy the denominator row
                to SBUF (stays on partition 64), tiny SBUF->SBUF DMA shifts
                it to partition 0, gpsimd partition_broadcast spreads it over
                64 partitions, reciprocal + normalize-mul on the staging
                copy, then SBUF->SBUF DMAs pair-pack into outTs.
                """
                stage = stp.tile([64, 2, 512], fp32r, tag="st")
                den64 = rqp.tile([65, 1024], fp32, tag="rq")
                den0 = rqp.tile([1, 1024], fp32, tag="d0")
                rbt = rbp.tile([64, 2048], fp32, tag="rb")
                for a in range(2):
                    nc.vector.tensor_copy(stage[:, a, :], oab[a][0:64, :])
                    nc.vector.tensor_copy(
                        den64[64:65, ts(a, 512)], oab[a][64:65, :]
                    )
                nc.sync.dma_start(den0[:], den64[64:65, :])
                nc.gpsimd.partition_broadcast(rbt[:, 0:1024], den0[:])
                nc.vector.reciprocal_approx_fast(
                    rbt[:, 1024:2048], rbt[:, 0:1024]
                )
                for a in range(2):
                    nc.vector.tensor_tensor(
                        stage[:, a, :],
                        stage[:, a, :],
                        rbt[:, 1024 + a * 512 : 1024 + (a + 1) * 512].bitcast(
                            fp32r
                        ),
                        Mult,
                    )
                    nc.sync.dma_start(
                        outTs[a * 64 : (a + 1) * 64, hp, ts(sqc, 512)],
                        stage[:, a, :],
                    )

            def proj_pieces(sqc):
                """Output projection for one sqc as a list of emission thunks."""
                pieces = []
                for mm in range(4):
                    m = sqc * 4 + mm
                    for nn in range(2):
                        def piece(m=m, nn=nn):
                            ps = paux.tile([128, 512], fp32, tag="aux")
                            for hp in range(2):
                                nc.tensor.matmul(
                                    ps[:],
                                    outTs[:, hp, ts(m, 128)],
                                    wo_sb[:, hp, ts(nn, 512)],
                                    start=(hp == 0),
                                    stop=(hp == 1),
                                )
                            ot = otp.tile([128, 512], fp32, tag="ot")
                            nc.vector.tensor_copy(ot[:], ps[:])
                            nc.sync.dma_start(
                                out[m * 128 : (m + 1) * 128, ts(nn, 512)],
                                ot[:],
                            )
                        pieces.append(piece)
                return pieces

            def emit_group(hp, sqc, g, fillers=()):
                """16 k-blocks + epilogue; fillers run every other k-block."""
                oab = [
                    poab.tile([65, 512], fp32, tag="o", name=f"o{a}")
                    for a in range(2)
                ]
                fill = list(fillers)
                fi = 0
                for kb in range(16):
                    emit_kb(hp, sqc, kb, oab)
                    if kb % 2 == 1 and fi < len(fill):
                        fill[fi]()
                        fi += 1
                while fi < len(fill):
                    fill[fi]()
                    fi += 1
                endgroup(hp, sqc, oab, g)

            # ---------------- main emission --------------------------------
            # group (hp0, sqc0) is interleaved into phase 1: its k-block j
            # only needs kT/v for s-chunk j//4, which emit_sc(j//4) provides.
            oab00 = [
                poab.tile([65, 512], fp32, tag="o", name=f"o{a}")
                for a in range(2)
            ]
            for sc in range(4):
                emit_sc(sc, xc=xc0 if sc == 0 else None)
                if sc == 1:
                    nc.sync.dma_start(
                        wo_sb[:], wo[:].rearrange("(hp p) n -> p hp n", p=128)
                    )
                for kb in range(4 * sc, 4 * sc + 4):
                    emit_kb(0, 0, kb, oab00)
            endgroup(0, 0, oab00, 0)

            emit_group(1, 0, 1)
            emit_group(0, 1, 2, proj_pieces(0))
            emit_group(1, 1, 3)
            emit_group(0, 2, 4, proj_pieces(1))
            emit_group(1, 2, 5)
            emit_group(0, 3, 6, proj_pieces(2))
            emit_group(1, 3, 7)
            for piece in proj_pieces(3):
                piece()

    nc.compile()
    return nc


def _get_nc():
    if "nc" not in _state:
        _state["nc"] = _build()
    return _state["nc"]


def _prep_core_inputs(inputs, Wq, bq, Wk, bk, Wv, bv, Wo, bo):
    """Build the 8 per-core input dicts (host-side shard + transpose)."""
    in_maps = []
    xTs = [np.ascontiguousarray(inputs[b].T, dtype=np.float32) for b in range(2)]
    for c in range(NCORES):
        b, g = divmod(c, 4)
        cols = slice(g * GD, (g + 1) * GD)
        m = {
            "xT": xTs[b],
            "wq": np.ascontiguousarray(Wq[:, cols], dtype=np.float32),
            "wk": np.ascontiguousarray(Wk[:, cols], dtype=np.float32),
            "wv": np.ascontiguousarray(Wv[:, cols], dtype=np.float32),
            "bq": np.ascontiguousarray(bq[cols], dtype=np.float32),
            "bk": np.ascontiguousarray(bk[cols], dtype=np.float32),
            "bv": np.ascontiguousarray(bv[cols], dtype=np.float32),
            "wo": np.ascontiguousarray(Wo[cols, :], dtype=np.float32),
        }
        in_maps.append(m)
    return in_maps


def run(inputs, Wq, bq, Wk, bk, Wv, bv, Wo, bo, trace=False):
    from concourse.bass_utils import run_bass_kernel_spmd

    nc = _get_nc()
    in_maps = _prep_core_inputs(inputs, Wq, bq, Wk, bk, Wv, bv, Wo, bo)
    res = run_bass_kernel_spmd(
        nc, in_maps, core_ids=list(range(NCORES)), trace=trace
    )
    out = np.zeros((2, S, D), np.float32)
    for c in range(NCORES):
        out[c // 4] += res.results[c]["out"]
    out += np.asarray(bo, np.float32)
    return out, res


def kernel(inputs, Wq, bq, Wk, bk, Wv, bv, Wo, bo):
    out, _ = run(
        np.asarray(inputs, np.float32),
        np.asarray(Wq, np.float32), np.asarray(bq, np.float32),
        np.asarray(Wk, np.float32), np.asarray(bk, np.float32),
        np.asarray(Wv, np.float32), np.asarray(bv, np.float32),
        np.asarray(Wo, np.float32), np.asarray(bo, np.float32),
    )
    return out
